# revision 1
# baseline (speedup 1.0000x reference)
"""Trainium2 Bass kernel for nn_LiquidNeuronEncoder.

The reference module (faithful to the torch source) never updates the hidden
state inside its time loop, so the output depends only on the LAST timestep:

    x     = input_seq[:, -1, 0]                     # [S]
    delta = input_seq[:, -1, 1]                     # [S]
    pre   = x * in_w[h] + (in_b[h] + wh_b[h])       # [S, H]
    dh    = tanh(pre) / tau[h]
    h     = delta[:, None] * dh                     # [S, H]
    out   = tanh(h @ out_w.T + out_b)               # [S, L]

Sharding: pure data parallel along S across 8 cores (1024 sequences each,
stacked as 2 chunks of 512 on the 128 partitions, h on partitions).

Design (vs the replicate-x baseline, 15.7us -> ~11.8us): minimize DMA bytes
and latency, keep every serial stage tight.

  inputs per core (~72KB total instead of 559KB):
    xd [2, 1280] bf16 (Scalar HWDGE, issued first - PE blocks on it; only
        2 descriptors): row c: cols 0:512 x chunk c, 512:1024 delta chunk
        c, 1024:1152 lhsT_x ([in_w|0] / [0|in_w]), 1152:1280 lhsT_d (ones)
    wp [128, 131] f32 (Sync HWDGE, in parallel): cols 0:128 block-diag
        out_w.T/tau (f32r matmul keeps error at the bf16-input level; bf16
        here doubles it past the 2e-2 gate), col 128 in_b+wh_b tiled,
        col 129 out_b tiled, col 130 zeros.
    (Descriptor rule: >=512B per partition row - a [128,4] f32 "misc" DMA
    has 16B descriptors and measures ~3us on HW; never split small columns
    into their own DMA. Engine rule: a cold HWDGE queue costs ~1.2us on its
    first issue either as a Sync pre-drain or inside the first DMA slice;
    gpsimd's SWDGE drain lands even later.)

  device program (single basic block - no bass Block, so no branches; init
  barrier + const memsets + dead register preamble stripped; cross-engine
  waits attached directly to consumer instructions so the walrus scheduler
  cannot hoist them above independent work, which it otherwise does; the
  auto-inserted 1.3us ACT table load is moved after the Scalar DMA issue
  post-compile so it overlaps the input DMA):
    PE : ps_pre = lhsT_x.T @ x      (K=2 bf16 outer product: x bcast * in_w)
         ps_db  = lhsT_d.T @ delta  (K=2 bf16: delta bcast across partitions)
         ps_out = w2r.T @ hn        (K=128 f32r, block-diag routes chunks)
    ACT: dh   = tanh(ps_pre + bc)   (per-partition bias)
         outT = tanh(ps_out + out_b) -> bf16
    DVE: w2r = f32r(wp); hn = dh * ps_db -> f32r
    Sync: output DMA (HWDGE). No completion wait: the NEFF epilogue
         (per-engine drains + all-engine barrier + serial sem clears, ~4us)
         far outlasts the ~1.5us the transfer still has in flight, and the
         next execution's first write to outT (ACT2) is ~8us in.
    (kv_writeback + trigger_dma descriptor pre-generation was tried and
    reverted: loading the gpsimd library takes ~7us in the background, so
    the Q7 prep cannot run any earlier than the trigger needs it.)

  output per core: [128, 512] bf16 (128KB); host converts to f32 and
  un-stacks the two chunks.
"""

import numpy as np
from contextlib import ExitStack

import concourse.bacc as bacc
from concourse import mybir
from concourse.bass_utils import run_bass_kernel_spmd

S, T, D = 8192, 2048, 2
H, L = 64, 64
NCORES = 8
SC = S // NCORES          # 1024 sequences per core
CH = 512                  # sequences per stacked chunk
NCH = SC // CH            # 2

_F32 = mybir.dt.float32
_F32R = mybir.dt.float32r
_BF16 = mybir.dt.bfloat16

XD_COLS = 2 * CH + 2 * (2 * H)   # 1024 xd | 128 lhsT_x | 128 lhsT_d = 1280
WP_COLS = 2 * H + 3              # 128 w2blk | bc | out_b | zeros = 131

STRIP_INIT_BARRIER = True  # drop the post-init all-engine barrier (the NEFF
                           # preamble's own $S[2] barrier already separates
                           # executions, and the epilogue clears our sems)
STRIP_ENGINE_PREAMBLE = True  # drop the per-engine InstRegisterMove +
                              # InstTPBBaseLd preamble (the ~1.2us TENSOR_LOAD
                              # per engine); nothing in this kernel reads the
                              # loaded registers - verified by rel-err on HW

_nc_cache = None


def _strip_framework_fat(nc):
    """Drop framework preamble instructions this kernel never needs:
    - the const-AP memsets (nothing reads them; ACT biases are APs)
    - the post-init all-engine barrier (drains + barrier_* EventSemaphores);
      data ordering is fully carried by this kernel's own semaphores, and
      the NEFF-level preamble/epilogue barriers separate executions."""
    bb = nc.m.functions[0].blocks[0]
    kept = []
    for i in bb.instructions:
        tn = type(i).__name__
        if tn == "InstMemset":
            continue
        if STRIP_INIT_BARRIER and tn == "InstDrain":
            continue
        if STRIP_INIT_BARRIER and tn == "InstEventSemaphore" and i.name.startswith(
            "barrier_"
        ):
            continue
        if STRIP_ENGINE_PREAMBLE and tn in ("InstRegisterMove", "InstTPBBaseLd"):
            continue
        kept.append(i)
    bb.instructions[:] = kept


def _move_act_table_load_after_dmas(nc):
    """insert_act_table_loads hoists the 1.3us InstLoadActFuncSet to the top
    of the Scalar stream, where it hogs the sequencer and delays the two
    Scalar-issued input DMAs by ~1us. Move it after the last Scalar DMACopy
    (it only needs to precede the first InstActivation)."""
    bb = nc.m.functions[0].blocks[0]
    insts = bb.instructions
    load_idx = last_dma_idx = None
    for idx, i in enumerate(insts):
        if i.engine != mybir.EngineType.Activation:
            continue
        tn = type(i).__name__
        if tn == "InstLoadActFuncSet":
            load_idx = idx
        elif tn == "InstDMACopy":
            last_dma_idx = idx
        elif tn == "InstActivation":
            break
    if load_idx is None:
        return
    if last_dma_idx is not None and load_idx < last_dma_idx:
        load = insts.pop(load_idx)
        insts.insert(last_dma_idx, load)  # list shifted left by the pop


def _build_raw():
    nc = bacc.Bacc("TRN2", target_bir_lowering=False, debug=False)
    xd_d = nc.dram_tensor("xd", [2, XD_COLS], _BF16, kind="ExternalInput")
    wp_d = nc.dram_tensor("wp", [2 * H, WP_COLS], _F32, kind="ExternalInput")
    out_d = nc.dram_tensor("out", [2 * H, CH], _BF16, kind="ExternalOutput")

    with ExitStack() as ctx:
        xd_s = ctx.enter_context(nc.sbuf_tensor("xd_s", [2, XD_COLS], _BF16)).ap()
        wp_s = ctx.enter_context(
            nc.sbuf_tensor("wp_s", [2 * H, WP_COLS], _F32)
        ).ap()
        w2r = ctx.enter_context(nc.sbuf_tensor("w2r", [2 * H, 2 * H], _F32R)).ap()
        dh = ctx.enter_context(nc.sbuf_tensor("dh", [2 * H, CH], _F32)).ap()
        hn = ctx.enter_context(nc.sbuf_tensor("hn", [2 * H, CH], _F32R)).ap()
        outT = ctx.enter_context(nc.sbuf_tensor("outT", [2 * H, CH], _BF16)).ap()
        ps_pre = ctx.enter_context(nc.psum_tensor("ps_pre", [2 * H, CH], _F32)).ap()
        ps_db = ctx.enter_context(nc.psum_tensor("ps_db", [2 * H, CH], _F32)).ap()
        ps_out = ctx.enter_context(nc.psum_tensor("ps_out", [2 * H, CH], _F32)).ap()

        dX = ctx.enter_context(nc.semaphore("dX"))
        dW = ctx.enter_context(nc.semaphore("dW"))
        cC = ctx.enter_context(nc.semaphore("cC"))
        dO = ctx.enter_context(nc.semaphore("dO"))

        x_rhs = xd_s[:, 0:CH]
        d_rhs = xd_s[:, CH : 2 * CH]
        lhsT_x = xd_s[:, 2 * CH : 2 * CH + 2 * H]
        lhsT_d = xd_s[:, 2 * CH + 2 * H : XD_COLS]
        w2f = wp_s[:, 0 : 2 * H]
        bc_ap = wp_s[:, 2 * H : 2 * H + 1]
        ob_ap = wp_s[:, 2 * H + 1 : 2 * H + 2]

        # cC chain: mm1=1, mm2=2, ACT1=3, TT=4, mm3=5, ACT2=6

        # --- Scalar: xd DMA (PE blocks on it), then the two tanhs ----------
        # (gpsimd SWDGE and Sync both tested slower for this: gpsimd's
        # preamble drain lands late, and Sync must carry wp first)
        nc.scalar.dma_start(out=xd_s, in_=xd_d[:, :]).then_inc(dX, 16)
        # ACT1 carries the dW wait itself; the cC wait rides a preceding
        # EventSem. (The reversed arrangement - EventSem on the DMA sems,
        # cC on ACT1 - measured ~150ns slower across disjoint sample sets.)
        nc.scalar.wait_ge(cC, 1)
        nc.scalar.activation(
            out=dh,
            in_=ps_pre,
            func=mybir.ActivationFunctionType.Tanh,
            bias=bc_ap,
            scale=1.0,
        )._wait_ge(dW, 16).then_inc(cC, 1)
        nc.scalar.activation(
            out=outT,
            in_=ps_out,
            func=mybir.ActivationFunctionType.Tanh,
            bias=ob_ap,
            scale=1.0,
        )._wait_ge(cC, 5).then_inc(cC, 1)
        # Output DMA behind ACT2 on the same engine (no cross-engine sem
        # hop). The cC gate is required for correctness: the Scalar SEQ runs
        # ahead of the ACT pipe, so an ungated copy races ACT2's commit
        # (measured margin ~450ns typical, but the fastest observed issue
        # slice composed with the slowest ACT2 goes negative - not worth the
        # ~4% for silent-corruption risk). Splitting into two parallel
        # gated halves on Scalar+Sync also loses: each half still saturates
        # the shared 16-engine DMA pool, so the transfers serialize and the
        # extra dge setup lands the last packet ~240ns later.
        nc.scalar.dma_start(out=out_d[:, :], in_=outT)._wait_ge(cC, 6).then_inc(
            dO, 16
        )

        # --- PE: broadcasts via K=2 outer products, then the big matmul ----
        nc.tensor.matmul(ps_pre, lhsT_x, x_rhs, start=True, stop=True)._wait_ge(
            dX, 16
        ).then_inc(cC, 1)
        nc.tensor.matmul(ps_db, lhsT_d, d_rhs, start=True, stop=True).then_inc(
            cC, 1
        )
        nc.tensor.matmul(ps_out, w2r, hn, start=True, stop=True)._wait_ge(
            cC, 4
        ).then_inc(cC, 1)

        # --- DVE: w2 rounding cast (early, off critical path), delta fold --
        nc.vector.tensor_copy(w2r, w2f)._wait_ge(dW, 16)
        nc.vector.tensor_mul(hn, dh, ps_db)._wait_ge(cC, 3).then_inc(cC, 1)

        # --- Sync: wp in (parallel with xd on gpsimd), output DMA out.
        # No completion wait on the output DMA: the NEFF epilogue (per-engine
        # drains + all-engine barrier + serial sem clears, ~4us) far outlasts
        # the ~1.5us the transfer still has in flight, and the next
        # execution's first write to outT (ACT2) is ~8us in. --------------
        nc.sync.dma_start(out=wp_s, in_=wp_d[:, :]).then_inc(dW, 16)

        nc.all_engine_barrier = lambda *a, **k: None

    _strip_framework_fat(nc)
    nc.compile()
    _move_act_table_load_after_dmas(nc)
    return nc


def _prep_inputs(input_seq, in_w, in_b, wh_w, wh_b, tau, out_w, out_b):
    import ml_dtypes

    f32 = lambda a: np.asarray(a, dtype=np.float32)
    last = f32(np.asarray(input_seq)[:, -1, :])        # [S, 2]
    xl = np.ascontiguousarray(last[:, 0])              # [S]
    dl = np.ascontiguousarray(last[:, 1])              # [S]

    in_w = f32(in_w).reshape(H)
    bc = f32(in_b) + f32(wh_b)                         # [H]
    obf = f32(out_b)
    w2base = f32(out_w).T / f32(tau).reshape(H, 1)     # [H, L]

    wp = np.zeros((2 * H, WP_COLS), dtype=np.float32)
    wp[0:H, 0:H] = w2base
    wp[H : 2 * H, H : 2 * H] = w2base
    wp[:, 2 * H] = np.tile(bc, 2)
    wp[:, 2 * H + 1] = np.tile(obf, 2)

    lx = np.zeros((2, 2 * H), dtype=np.float32)
    lx[0, 0:H] = in_w
    lx[1, H : 2 * H] = in_w
    ld = np.zeros((2, 2 * H), dtype=np.float32)
    ld[0, 0:H] = 1.0
    ld[1, H : 2 * H] = 1.0

    in_maps = []
    for i in range(NCORES):
        xs = xl[i * SC : (i + 1) * SC]
        ds = dl[i * SC : (i + 1) * SC]
        xd = np.empty((2, XD_COLS), dtype=np.float32)
        xd[0, 0:CH] = xs[0:CH]
        xd[1, 0:CH] = xs[CH:SC]
        xd[0, CH : 2 * CH] = ds[0:CH]
        xd[1, CH : 2 * CH] = ds[CH:SC]
        xd[:, 2 * CH : 2 * CH + 2 * H] = lx
        xd[:, 2 * CH + 2 * H : XD_COLS] = ld
        in_maps.append({"xd": xd.astype(ml_dtypes.bfloat16), "wp": wp})
    return in_maps


def _unshard_one(r):
    """[128, 512] bf16 core output -> [1024, 64] f32: partition p=(c*64+l),
    col j holds out[s = c*512 + j, l]."""
    a = np.asarray(r).astype(np.float32).reshape(NCH, H, CH)
    return np.ascontiguousarray(a.transpose(0, 2, 1).reshape(SC, L))


def _get_nc():
    global _nc_cache
    if _nc_cache is None:
        _nc_cache = _build_raw()
    return _nc_cache


def _run(in_maps, trace=False, **kwargs):
    nc = _get_nc()
    return run_bass_kernel_spmd(
        nc, in_maps, core_ids=list(range(NCORES)), trace=trace, **kwargs
    )


def kernel(**inputs):
    in_maps = _prep_inputs(**inputs)
    res = _run(in_maps)
    out = np.empty((S, L), dtype=np.float32)
    for i in range(NCORES):
        out[i * SC : (i + 1) * SC] = _unshard_one(res.results[i]["out"])
    return out

